# revision 40
# baseline (speedup 1.0000x reference)
"""Causal self-attention (B=4, T=2048, C=1024, 16 heads) on 8 trn2 NeuronCores.

Sharding: core (b, hg) handles batch b (4) x head-group hg (2 groups of 8 heads).
Each core computes QKV projection for its batch restricted to its 8 heads,
flash-style causal attention, and the output projection restricted to its
heads' rows of w_proj -> a partial [T, C] output. Host sums the two partials
per batch (tensor-parallel unshard) and concatenates batches.

Key layout choices (all bf16 matmul inputs, fp32 PSUM accumulation):
  - x is fed pre-transposed per batch: xT [C, T], so Q^T/K^T [d, t] come
    straight out of the QKV matmuls (lhsT = w slice, rhs = xT).
  - Scores are computed TRANSPOSED: S^T[tk, q] with lhsT = K^T chunk,
    rhs = Q^T chunk. The two heads of a pair sit at partitions 0-63 / 64-127
    so their K=64 matmuls tile-pack and run concurrently in the PE array.
  - V carries an appended ones-column, so the A@V matmul (lhsT=[V|1],
    rhs=P^T) yields y~^T = [64 weighted-V rows | row-sum row] per q-chunk.
  - Softmax normalize WITHOUT transposes: ACT ln on the PSUM sums row
    (fp16), a K=1 PE matmul broadcasts ln(l) across partitions, ACT
    exp(-.) turns it into a bf16 1/l broadcast, and one DVE multiply
    produces normalized y^T directly. Ln/Exp share one ACT table set (the
    cached activation-table dict is steered to natural_log_exp_and_others,
    otherwise the table-load pass thrashes ~1.3us DMAs per switch). Even
    heads write straight into the y^T buffer partitions 0-63; odd heads go
    via a staging tile + SBUF->SBUF DMA into partitions 64-127.
  - y^T accumulates in-place over the Q^T buffer (dead after its pair's
    scores), feeding the output projection with K=128 contraction chunks.
  - Causal masking: block-skipping, matmul column ranges narrowed to the
    valid q-range on diagonal slots, one merged exp per diagonal slot over
    both heads ([2, n] strided AP), and a single [128,2,128] staircase-mask
    multiply per diagonal slot.
  - Next pair's QKV projection matmuls are interleaved into the attention
    c-loop as PE filler while ScalarE works through the exps; pair 3
    instead interleaves the PREVIOUS chunk's output projection (so proj
    never waits on the in-flight normalize), with the last chunk's
    projection as the tail.
  - A burst of dummy K=1 matmuls spans the initial input-DMA wait to keep
    the PE's HAM activity window busy (first real matmuls start at 2.4GHz
    instead of the cold 1.2GHz ramp).
"""

import numpy as np
import ml_dtypes

B, T, C, H, D = 4, 2048, 1024, 16, 64
P = 128
TC = T // P          # 16 t-chunks of 128
KC = C // P          # 8 contraction chunks of 128
NPAIR = 4            # head pairs per core (8 local heads)
SCALE = 0.125        # 1/sqrt(64)

_CACHE = {}
LAST_RESULT = None   # BassKernelResults of the most recent run (for test.py)

BF16 = ml_dtypes.bfloat16


def _build_program():
    import concourse.tile as tile
    import concourse.mybir as mybir
    from concourse import bacc

    dt = mybir.dt
    AF = mybir.ActivationFunctionType
    ALU = mybir.AluOpType

    nc = bacc.Bacc("TRN2", target_bir_lowering=False, debug=False, num_devices=8)

    # This kernel's only ACT functions are Exp and Ln. The table-load pass
    # picks a set per function greedily, which thrashes between
    # `exp_and_others` and `natural_log` (a ~1.3us table DMA per switch,
    # dozens per kernel). Steer it to the one set that holds both by
    # removing the single-function alternatives from the cached table dict.
    from concourse.hw_specs import get_activation_tables
    AFt = mybir.ActivationFunctionType
    tabs = get_activation_tables(nc.m.arch)
    if "natural_log_exp_and_others" in tabs:
        # keep dict keys intact (act_func_set_id is an index into
        # act_info.json) -- just stop advertising Exp/Ln in other sets
        for k in tabs:
            if k != "natural_log_exp_and_others":
                tabs[k].discard(AFt.Exp)
                tabs[k].discard(AFt.Ln)

    # ---- DRAM I/O ----
    xT_d = nc.dram_tensor("xT", [C, T], dt.bfloat16, kind="ExternalInput").ap()
    wqk_d = nc.dram_tensor("wqk", [C, 1024], dt.bfloat16, kind="ExternalInput").ap()
    wv_d = nc.dram_tensor("wv", [C, 512], dt.bfloat16, kind="ExternalInput").ap()
    wproj_d = nc.dram_tensor("wproj", [512, C], dt.bfloat16, kind="ExternalInput").ap()
    bqk_d = nc.dram_tensor("bqk", [P, 8], dt.float32, kind="ExternalInput").ap()
    bv_d = nc.dram_tensor("bv", [P, 512], dt.bfloat16, kind="ExternalInput").ap()
    bproj_d = nc.dram_tensor("bproj", [P, C], dt.bfloat16,
                             kind="ExternalInput").ap()
    dmask2_d = nc.dram_tensor("dmask2", [P, 2 * P], dt.bfloat16,
                              kind="ExternalInput").ap()
    out_d = nc.dram_tensor("out", [T, C], dt.bfloat16, kind="ExternalOutput").ap()

    with tile.TileContext(nc) as tc:
        with (
            tc.tile_pool(name="const", bufs=1) as cp,
            tc.tile_pool(name="outp", bufs=8) as op_pool,
            tc.tile_pool(name="lnp", bufs=4) as ln_pool,
            tc.tile_pool(name="linvp", bufs=4) as linv_pool,
            tc.tile_pool(name="ystg", bufs=3) as ystg_pool,
            tc.tile_pool(name="psqk", bufs=2, space="PSUM") as psqk_pool,
            tc.tile_pool(name="psmm", bufs=4, space="PSUM") as psmm_pool,
        ):
            # ---- static SBUF tensors ----
            xT_s = cp.tile([P, KC, T], dt.bfloat16, name="xT_s")
            wqk_s = cp.tile([P, KC, 1024], dt.bfloat16, name="wqk_s")
            wv_s = cp.tile([P, KC, 512], dt.bfloat16, name="wv_s")
            wproj_s = cp.tile([P, 4, C], dt.bfloat16, name="wproj_s")
            bqk_s = cp.tile([P, 8], dt.float32, name="bqk_s")
            bv_s = cp.tile([P, 512], dt.bfloat16, name="bv_s")
            bproj_s = cp.tile([P, C], dt.bfloat16, name="bproj_s")
            dmask2_s = cp.tile([P, 2, P], dt.bfloat16, name="dmask2_s")
            ones16_s = cp.tile([P, P], dt.float16, name="ones16_s")
            qt_s = cp.tile([P, NPAIR, T], dt.bfloat16, name="qt_s")  # later y^T
            kt_s = cp.tile([P, NPAIR, T], dt.bfloat16, name="kt_s")
            v_s = cp.tile([P, TC, 8, 66], dt.bfloat16, name="v_s")   # [t,tc,h,V|1]
            pt_s = cp.tile([P, 12, 2, 512], dt.bfloat16, name="pt_s")   # off-diag
            ptd_s = cp.tile([P, 4, 2, 512], dt.bfloat16, name="ptd_s")  # diag

            # ---- input DMAs (first-compute tensors first, split for
            # parallel queues) ----
            xT_src = xT_d.rearrange("(o p) t -> p o t", p=P)
            wv_src = wv_d.rearrange("(o p) m -> p o m", p=P)
            wqk_src = wqk_d.rearrange("(o p) m -> p o m", p=P)
            nc.sync.dma_start(wv_s[:, 0:4, :], wv_src[:, 0:4, :])
            nc.sync.dma_start(xT_s[:, :, 0:256], xT_src[:, :, 0:256])
            nc.sync.dma_start(wv_s[:, 4:8, :], wv_src[:, 4:8, :])
            nc.sync.dma_start(bv_s[:], bv_d)
            nc.sync.dma_start(xT_s[:, :, 256:512], xT_src[:, :, 256:512])
            nc.sync.dma_start(xT_s[:, :, 512:768], xT_src[:, :, 512:768])
            nc.sync.dma_start(wqk_s[:, 0:4, :], wqk_src[:, 0:4, :])
            nc.sync.dma_start(xT_s[:, :, 768:1024], xT_src[:, :, 768:1024])
            nc.sync.dma_start(wqk_s[:, 4:8, :], wqk_src[:, 4:8, :])
            nc.sync.dma_start(bqk_s[:], bqk_d)
            for q8 in range(4, 8):
                nc.sync.dma_start(xT_s[:, :, 256 * q8:256 * (q8 + 1)],
                                  xT_src[:, :, 256 * q8:256 * (q8 + 1)])
            nc.sync.dma_start(dmask2_s[:],
                              dmask2_d.rearrange("p (a b) -> p a b", a=2))
            nc.sync.dma_start(wproj_s[:], wproj_d.rearrange("(o p) m -> p o m", p=P))
            nc.sync.dma_start(bproj_s[:], bproj_d)

            # ones column of V~; ones row for the ln(l) broadcast matmul; zero
            # the diag P^T buffer once (sub-diagonal regions are never read)
            nc.vector.memset(ones16_s[:], 1.0)
            nc.vector.memset(v_s[:, :, :, 64:65], 1.0)
            nc.vector.memset(ptd_s[:], 0.0)

            # PE warm-up: dummy serialized matmuls spanning the initial DMA
            # wait keep the HAM activity window busy so the first real
            # matmuls run at 2.4 GHz instead of the cold 1.2 GHz ramp.
            warm = psmm_pool.tile([P, P], dt.float32, name="warm", tag="mm")
            for _ in range(66):
                nc.tensor.matmul(warm[0:64, :], ones16_s[64:65, 0:64],
                                 ones16_s[64:65, :], start=True, stop=True)

            # ---- V projection: v[t, ch] for all 8 heads (512 cols) ----
            for tcx in range(TC):
                psv = psmm_pool.tile([P, 512], dt.float32, name="psv", tag="mm")
                for k in range(KC):
                    nc.tensor.matmul(psv[:, :],
                                     xT_s[:, k, P * tcx:P * (tcx + 1)],
                                     wv_s[:, k, :],
                                     start=(k == 0), stop=(k == KC - 1))
                nc.vector.tensor_add(
                    out=v_s[:, tcx, :, 0:64],
                    in0=psv[:, :].rearrange("a (h d) -> a h d", h=8),
                    in1=bv_s[:, :].rearrange("a (h d) -> a h d", h=8),
                )

            # ---- helper emitters ----
            def qkproj_chunk(m, t4):
                """One [128 out-ch, 512 t] tile of the Q^T/K^T projection."""
                dst = qt_s if m < 4 else kt_s
                psq = psmm_pool.tile([P, 512], dt.float32, name="psq", tag="mm")
                for k in range(KC):
                    nc.tensor.matmul(psq[:, :],
                                     wqk_s[:, k, P * m:P * (m + 1)],
                                     xT_s[:, k, 512 * t4:512 * (t4 + 1)],
                                     start=(k == 0), stop=(k == KC - 1))
                nc.vector.tensor_scalar(
                    out=dst[:, m % 4, 512 * t4:512 * (t4 + 1)],
                    in0=psq[:, :], scalar1=bqk_s[:, m:m + 1], scalar2=None,
                    op0=ALU.add)

            def proj_chunk(tcx):
                for co in range(2):
                    psp = psmm_pool.tile([P, 512], dt.float32, name="psp", tag="mm")
                    for cc in range(4):
                        nc.tensor.matmul(psp[:, :],
                                         qt_s[:, cc, P * tcx:P * (tcx + 1)],
                                         wproj_s[:, cc, 512 * co:512 * (co + 1)],
                                         start=(cc == 0), stop=(cc == 3))
                    ot = op_pool.tile([P, 512], dt.bfloat16, name="ot", tag="ot")
                    nc.vector.tensor_add(out=ot[:, :], in0=psp[:, :],
                                         in1=bproj_s[:, 512 * co:512 * (co + 1)])
                    nc.sync.dma_start(
                        out_d[P * tcx:P * (tcx + 1), 512 * co:512 * (co + 1)], ot[:, :])

            # ---- pair 0 projection upfront; later pairs interleave ----
            for m in (0, 4):
                for t4 in range(4):
                    qkproj_chunk(m, t4)

            for pair in range(NPAIR):
                nxt = ([(m, t4) for m in (pair + 1, 5 + pair) for t4 in range(4)]
                       if pair < NPAIR - 1 else [])
                for ci, c in enumerate(range(4)):   # q chunk of 512
                    for j in range(4 * c + 4):          # tk chunk (slot)
                        r = j - 4 * c                   # >= 0 on diagonal slots
                        q0 = P * r if r >= 0 else 0     # skip masked cols
                        psS = psqk_pool.tile([P, 2, 512], dt.float32, name="psS",
                                             tag="psqk")
                        for hh in (0, 1):
                            base = 64 * hh
                            nc.tensor.matmul(
                                psS[:, hh, q0:],
                                kt_s[base:base + 64, pair, P * j:P * (j + 1)],
                                qt_s[base:base + 64, pair,
                                     512 * c + q0:512 * (c + 1)],
                                start=True, stop=True)
                        # exp( S^T * scale ), fp32 psum -> bf16 sbuf
                        if r < 0:
                            nc.scalar.activation(pt_s[:, j, :, :], psS[:, :, :],
                                                 AF.Exp, scale=SCALE)
                        else:
                            nc.scalar.activation(ptd_s[:, r, :, q0:],
                                                 psS[:, :, q0:],
                                                 AF.Exp, scale=SCALE)
                            # staircase mask on the true diagonal block
                            nc.vector.tensor_tensor(
                                out=ptd_s[:, r, :, q0:q0 + P],
                                in0=ptd_s[:, r, :, q0:q0 + P],
                                in1=dmask2_s[:, :, :], op=ALU.mult)

                    # PE filler while ScalarE works through the exps: for
                    # pairs 0-2 the next pair's Q^T/K^T projection; for
                    # pair 3 the output projection of the PREVIOUS chunk
                    # (so it never waits on this chunk's normalize).
                    if pair < NPAIR - 1:
                        for (m, t4) in nxt[2 * ci:2 * ci + 2]:
                            qkproj_chunk(m, t4)
                    elif c >= 1:
                        for qi_loc in range(4):
                            proj_chunk(4 * (c - 1) + qi_loc)

                    # [V | 1]^T @ P^T per head: y~^T [65, 512] with the
                    # softmax denominator in row 64, both heads in one
                    # 2-bank PSUM tile. Normalize without transposes:
                    # one ln(l) over both sums rows (ACT), K=1 matmul
                    # broadcast per head, exp(-.) back to bf16 (Ln/Exp
                    # share one ACT table set), one DVE multiply per head.
                    nj = 4 * c + 4
                    psyts, lnrows = [], []
                    for hh in (0, 1):
                        h = 2 * pair + hh
                        psyt = psmm_pool.tile([P, 512], dt.float32,
                                              name="psyt", tag="mm")
                        for j in range(nj):
                            r = j - 4 * c
                            if r < 0:
                                rhs = pt_s[:, j, hh, :]
                                out = psyt[0:65, :]
                            else:
                                # diagonal slot: only columns q >= 128r live
                                rhs = ptd_s[:, r, hh, P * r:]
                                out = psyt[0:65, P * r:]
                            nc.tensor.matmul(
                                out, v_s[:, j, h, 0:65], rhs,
                                start=(j == 0), stop=(j == nj - 1))
                        lnrow = ln_pool.tile([P, 512], dt.float16,
                                             name="lnrow", tag="lnrow")
                        with nc.allow_low_precision(
                                reason="fp16 ln(l): abs err <= 26*2^-12"):
                            nc.scalar.activation(lnrow[64:65, :],
                                                 psyt[64:65, :], AF.Ln)
                        psyts.append(psyt)
                        lnrows.append(lnrow)
                    psls = []
                    for hh in (0, 1):
                        psl = psmm_pool.tile([P, 512], dt.float32, name="psl",
                                            tag="mm")
                        nc.tensor.matmul(
                            psl[0:64, :], ones16_s[64:65, 0:64],
                            lnrows[hh][64:65, :], start=True, stop=True)
                        psls.append(psl)
                    for hh in (0, 1):
                        linvb = linv_pool.tile([P, 512], dt.bfloat16,
                                               name="linvb", tag="linvb")
                        nc.scalar.activation(linvb[0:64, :], psls[hh][0:64, :],
                                             AF.Exp, scale=-1.0)
                        if hh == 0:
                            ydst = qt_s[0:64, pair, 512 * c:512 * (c + 1)]
                            nc.vector.tensor_tensor(
                                out=ydst, in0=psyts[hh][0:64, :],
                                in1=linvb[0:64, :], op=ALU.mult)
                        else:
                            ystg = ystg_pool.tile([P, 512], dt.bfloat16,
                                                  name="ystg", tag="ystg")
                            nc.vector.tensor_tensor(
                                out=ystg[0:64, :], in0=psyts[hh][0:64, :],
                                in1=linvb[0:64, :], op=ALU.mult)
                            nc.sync.dma_start(
                                qt_s[64:128, pair, 512 * c:512 * (c + 1)],
                                ystg[0:64, :])

            # output projection tail for the last chunk of pair 3
            for qi_loc in range(4):
                proj_chunk(12 + qi_loc)

    nc.compile()
    return nc


def _prep_inputs(x, w_attn, b_attn, w_proj, b_proj):
    """Host-side shard prep: per-core input dicts (core ci = b*2 + hg)."""
    x = np.asarray(x, dtype=np.float32)
    w_attn = np.asarray(w_attn, dtype=np.float32)
    b_attn = np.asarray(b_attn, dtype=np.float32)
    w_proj = np.asarray(w_proj, dtype=np.float32)
    b_proj = np.asarray(b_proj, dtype=np.float32)

    # diagonal staircase mask [tk, q]: valid iff q >= tk; duplicated per head
    dmask = (np.arange(P)[None, :] >= np.arange(P)[:, None]).astype(BF16)
    dmask2 = np.concatenate([dmask, dmask], axis=1)          # [P, 2P]

    in_maps = []
    for b in range(B):
        xT = np.ascontiguousarray(x[b].T).astype(BF16)       # [C, T]
        for hg in range(2):
            lo = hg * 512
            wqk = np.concatenate(
                [w_attn[:, lo:lo + 512], w_attn[:, 1024 + lo:1024 + lo + 512]],
                axis=1).astype(BF16)                          # [C, 1024]
            wv = w_attn[:, 2048 + lo:2048 + lo + 512].astype(BF16)
            wproj = w_proj[lo:lo + 512, :].astype(BF16)       # [512, C]
            bqk = np.stack(
                [b_attn[lo + P * m:lo + P * (m + 1)] for m in range(4)] +
                [b_attn[1024 + lo + P * m:1024 + lo + P * (m + 1)] for m in range(4)],
                axis=1).astype(np.float32)                    # [128, 8]
            bv = np.broadcast_to(b_attn[2048 + lo:2048 + lo + 512],
                                 (P, 512)).astype(BF16)
            bp = b_proj if hg == 0 else np.zeros_like(b_proj)
            bproj = np.broadcast_to(bp, (P, C)).astype(BF16)
            in_maps.append({
                "xT": xT, "wqk": wqk, "wv": wv, "wproj": wproj,
                "bqk": np.ascontiguousarray(bqk), "bv": np.ascontiguousarray(bv),
                "bproj": np.ascontiguousarray(bproj),
                "dmask2": np.ascontiguousarray(dmask2),
            })
    return in_maps


def kernel(x, w_attn, b_attn, w_proj, b_proj):
    global LAST_RESULT
    from concourse.bass_utils import run_bass_kernel_spmd

    if "nc" not in _CACHE:
        _CACHE["nc"] = _build_program()
    nc = _CACHE["nc"]

    in_maps = _prep_inputs(x, w_attn, b_attn, w_proj, b_proj)
    res = run_bass_kernel_spmd(nc, in_maps, core_ids=list(range(8)))
    LAST_RESULT = res

    out = np.zeros((B, T, C), dtype=np.float32)
    for b in range(B):
        out[b] = (res.results[2 * b]["out"].astype(np.float32) +
                  res.results[2 * b + 1]["out"].astype(np.float32))
    return out
